# revision 21
# baseline (speedup 1.0000x reference)
"""Trainium2 Bass kernel for DiffSelfAttention (B=1, T=2048, C=2048, 16 v-heads).

Sharding: tensor-parallel over heads across 8 NeuronCores. Core c owns
v-heads {2c, 2c+1} plus the matching q/k heads of both differential branches.
Each core computes its qkv slice, the attention for its 4 q/k heads, the
differential + per-head RMSNorm, and a partial projection
y_c = out_c @ w_proj[rows_c]. The host sums the 8 partials (unshard step).

Layout/strategy notes (v3):
  - Everything that feeds the PE is bf16 (x, w_qkv slices, q/k/v, exp(scores),
    w_proj, normed out). PSUM accumulation stays fp32, so only input rounding
    (~0.1% rms) enters; measured end-to-end rel err ~2e-3 vs the 2e-2 budget.
    bf16 also enables Fast Weight Load on LDWEIGHTS and 2x DVE modes.
  - q/k produced transposed ([d, T]); v natural ([T, d]); scores computed
    transposed ([tk, tq]) so probs@v needs no transposes anywhere.
  - Softmax divisions eliminated: RMSNorm is invariant to per-column positive
    scale, so o' = a1*r2 - lam*a2*r1 replaces a1/r1 - lam*a2/r2.
  - The attention inner loop is software-pipelined: scores(k+1) is emitted on
    the PE before pv(k), so the PE streams scores while ACT exps slab k.
  - exp-sum r is accumulated on the DVE (bf16 adds over the et ring) with one
    final ones-matmul for the cross-partition reduction; this removes 131k PE
    cycles and frees 2 PSUM banks vs accumulating r on the PE.
  - RMSNorm (exp(-0.5*ln(mean)) on ACT) and the output projection run
    per-512-column group; the projection matmuls/copies/DMAs of group g are
    emitted one step per slab-beat inside group g+1, hiding them in the
    PE/ACT slack and spreading the output DMA across the whole kernel.
  - ~36 ones@ones warm-up matmuls at kernel start keep the PE busy while the
    first weight DMAs land, so the HAM clock-gate releases (1.2->2.4 GHz)
    right as real work begins. A dummy exp prepays the ACT table load.
"""

import math

import numpy as np
import ml_dtypes

import concourse.bass as bass
import concourse.bacc as bacc
import concourse.mybir as mybir
import concourse.tile as tile

F32 = mybir.dt.float32
F32R = mybir.dt.float32r
BF16 = mybir.dt.bfloat16

T = 2048
C = 2048
N_HEAD = 16
H_DIM = 64
D2 = 2 * H_DIM  # 128 (v-head dim, also the RMS group size)
LAMBDA_INIT = 0.8 - 0.6 * math.exp(-0.3)
SCALE = 1.0 / math.sqrt(H_DIM)
P = 128
KSLABS = C // P  # 16 contraction slabs
TT = T // P  # 16 t-tiles
NCH = 512  # tq chunk = PSUM bank width in fp32
N_CORES = 8
DMAG = 2  # k-slab granularity of the startup weight/x DMA stream
NWARM = 36  # HAM warm-up matmuls

EXP = mybir.ActivationFunctionType.Exp
LOG = mybir.ActivationFunctionType.Ln
MULT = mybir.AluOpType.mult


def build(lam: float) -> bass.Bass:
    nc = bacc.Bacc("TRN2", target_bir_lowering=False, debug=False)

    xt_d = nc.dram_tensor("xt", [P, 4, KSLABS, NCH], BF16, kind="ExternalInput")
    wqk_d = nc.dram_tensor("wqk", [P, KSLABS, 4 * P], BF16, kind="ExternalInput")
    wv_d = nc.dram_tensor("wv", [P, KSLABS, 2 * D2], BF16, kind="ExternalInput")
    wp_d = nc.dram_tensor("wp", [P, 2, T], BF16, kind="ExternalInput")
    sv_d = nc.dram_tensor("sv", [P, 1], F32, kind="ExternalInput")
    y_d = nc.dram_tensor("y", [TT, P, T], BF16, kind="ExternalOutput")

    with tile.TileContext(nc) as tc:
        with tc.tile_pool(name="persist", bufs=1) as persist:
            sv = persist.tile([P, 1], F32)
            ones_f = persist.tile([P, P], F32)
            ones = persist.tile([P, P], BF16)
            actw = persist.tile([P, 1], F32)
            qk = persist.tile([P, 4, T], BF16)  # q1|q2|k1|k2, [d, T] layout
            vnat = persist.tile([P, TT, 2 * D2], BF16)  # v, [T, d] layout
            nc.sync.dma_start(out=sv, in_=sv_d[:])
            nc.vector.memset(ones_f, 1.0)
            nc.vector.tensor_copy(ones, ones_f)
            # prepay the exp table load while the PE warms up
            nc.scalar.activation(actw, ones_f[:, 0:1], EXP, scale=1.0)

            # ---------- phase 1: qkv projections ----------
            with tc.tile_pool(name="w1", bufs=1) as w1p, \
                 tc.tile_pool(name="xt", bufs=2) as xtp, \
                 tc.tile_pool(name="ps_qk", bufs=2, space="PSUM") as pqk, \
                 tc.tile_pool(name="ps_v", bufs=2, space="PSUM") as pvp:
                warm = pqk.tile([P, P], F32, tag="warm", bufs=1)
                for _ in range(NWARM):
                    nc.tensor.matmul(warm, ones, ones, start=True, stop=True)
                wqk = w1p.tile([P, KSLABS, 4 * P], BF16)
                wv = w1p.tile([P, KSLABS, 2 * D2], BF16)
                for n in range(T // NCH):  # 512-wide t chunks
                    xt = xtp.tile([P, KSLABS, NCH], BF16)
                    if n == 0:
                        # Stream wqk + x chunk 0 in small interleaved k-slab
                        # groups so the first matmul starts a few us in (one
                        # big DMA each would stall the PE ~40us at start).
                        for g in range(KSLABS // DMAG):
                            sl = slice(g * DMAG, (g + 1) * DMAG)
                            nc.sync.dma_start(out=wqk[:, sl, :], in_=wqk_d[:, sl, :])
                            nc.sync.dma_start(out=xt[:, sl, :], in_=xt_d[:, 0, sl, :])
                        nc.sync.dma_start(out=wv, in_=wv_d[:])
                    else:
                        nc.sync.dma_start(out=xt, in_=xt_d[:, n, :, :])
                    # k-major accumulation into 4 live psum banks: each
                    # arriving x/w k-slab is consumed immediately, so the
                    # chunk-0 DMA stream never stalls a psum chain.
                    pss = [pqk.tile([P, NCH], F32, tag=f"m{m}", bufs=1,
                                    name=f"psqk{m}") for m in range(4)]
                    for k in range(KSLABS):
                        for m in range(4):  # q1, q2, k1, k2
                            nc.tensor.matmul(
                                pss[m],
                                wqk[:, k, m * P:(m + 1) * P],
                                xt[:, k, :],
                                start=(k == 0),
                                stop=(k == KSLABS - 1),
                            )
                    for m in range(4):
                        nc.vector.tensor_copy(qk[:, m, n * NCH:(n + 1) * NCH], pss[m])
                    for t2 in range(NCH // P):  # t-tiles in this chunk
                        ps = pvp.tile([P, 2 * D2], F32)
                        for k in range(KSLABS):
                            nc.tensor.matmul(
                                ps,
                                xt[:, k, t2 * P:(t2 + 1) * P],
                                wv[:, k, :],
                                start=(k == 0),
                                stop=(k == KSLABS - 1),
                            )
                        nc.vector.tensor_copy(vnat[:, n * (NCH // P) + t2, :], ps)

            # ---------- phases 2+3, fused per 512-column group ----------
            with tc.tile_pool(name="wp", bufs=1) as wpp, \
                 tc.tile_pool(name="ps_s", bufs=2, space="PSUM") as psp, \
                 tc.tile_pool(name="ps_a", bufs=1, space="PSUM") as pap, \
                 tc.tile_pool(name="ps_r", bufs=1, space="PSUM") as rmsp, \
                 tc.tile_pool(name="ps_y", bufs=1, space="PSUM") as pyp, \
                 tc.tile_pool(name="exp", bufs=2) as ep, \
                 tc.tile_pool(name="keep", bufs=1) as kp, \
                 tc.tile_pool(name="ysb", bufs=3) as yp:
                wp = wpp.tile([P, 2, T], BF16)
                nc.sync.dma_start(out=wp, in_=wp_d[:])

                # Deferred projection steps: emitted one per slab-beat so the
                # proj matmuls ride in the attention loop's dependency slack
                # instead of serializing between groups.
                pending = []

                def emit_proj(q4):
                    on_t = on[q4]
                    for t2 in range(NCH // P):
                        ysb = yp.tile([P, T], BF16, tag="ysb", name=f"ysb{q4}{t2}")
                        for nchk in range(T // NCH):
                            def step(t2=t2, nchk=nchk, ysb=ysb, on_t=on_t, q4=q4,
                                     pool=None):
                                if pool is None:
                                    py = pyp.tile([P, NCH], F32, name="py")
                                else:
                                    py = pool.tile([P, NCH], F32, tag="s", name="py")
                                for vh in range(2):
                                    nc.tensor.matmul(
                                        py,
                                        on_t[:, vh, t2 * P:(t2 + 1) * P],
                                        wp[:, vh, nchk * NCH:(nchk + 1) * NCH],
                                        start=(vh == 0),
                                        stop=(vh == 1),
                                    )
                                nc.vector.tensor_copy(
                                    ysb[:, nchk * NCH:(nchk + 1) * NCH], py)
                                if nchk == T // NCH - 1:
                                    nc.sync.dma_start(
                                        out=y_d[q4 * (NCH // P) + t2], in_=ysb)
                            pending.append(step)

                on = {}
                for q4 in range(4):
                    c0 = q4 * NCH
                    a1u = None
                    r1l = None
                    opk = None
                    for br in range(2):
                        pa = pap.tile([P, 2, NCH], F32, tag="pa", name="pa")
                        racc = kp.tile([P, 2, NCH], BF16, tag=f"racc{br}",
                                       name=f"racc{br}")
                        racg = kp.tile([P, 2, NCH], BF16, tag=f"racg{br}",
                                       name=f"racg{br}")

                        def scores(k):
                            ps = psp.tile([P, 2, NCH], F32, tag="s", name="ps")
                            for vh in range(2):
                                rows = slice(vh * H_DIM, (vh + 1) * H_DIM)
                                nc.tensor.matmul(
                                    ps[:, vh, :],
                                    qk[rows, 2 + br, k * P:(k + 1) * P],
                                    qk[rows, br, c0:c0 + NCH],
                                    start=True,
                                    stop=True,
                                )
                            return ps

                        ps_cur = scores(0)
                        for k in range(TT):  # tk slabs
                            ps_nxt = scores(k + 1) if k + 1 < TT else None
                            et = ep.tile([P, 2, NCH], BF16, tag="er", bufs=14,
                                         name="et")
                            nc.scalar.activation(et, ps_cur, EXP, scale=SCALE)
                            for vh in range(2):
                                nc.tensor.matmul(
                                    pa[:, vh, :],
                                    vnat[:, k, vh * D2:(vh + 1) * D2],
                                    et[:, vh, :],
                                    start=(k == 0),
                                    stop=(k == TT - 1),
                                )
                            if k == 0:
                                nc.vector.tensor_copy(racc, et)
                            elif k % 3 == 2:
                                # every third slab's exp-sum accumulates on the
                                # otherwise-idle GpSimd to unload the DVE
                                if k == 2:
                                    nc.gpsimd.tensor_copy(racg, et)
                                else:
                                    nc.gpsimd.tensor_add(racg, racg, et)
                            else:
                                nc.vector.tensor_add(racc, racc, et)
                            if k % 2 == 1 and pending:
                                # one deferred proj step every other beat: the
                                # DVE can't absorb a copy every beat on top of
                                # the racc chain
                                pending.pop(0)()
                            ps_cur = ps_nxt
                        nc.vector.tensor_add(racc, racc, racg)
                        # keep-alive matmuls: the PE would otherwise idle here
                        # waiting for the racc chain + exp-stream restart, and
                        # >3.4us idle re-throttles the PE clock to 1.2 GHz for
                        # the next ~10us (HAM)
                        wt = rmsp.tile([P, NCH], F32, tag="r", name="wt")
                        for _ in range(12):
                            nc.tensor.matmul(wt[:, 0:P], ones, ones,
                                             start=True, stop=True)
                        # cross-partition reduce of the exp-sums (broadcasts
                        # the column sums to all 128 partitions). Lives in the
                        # dedicated 1-bank rms pool so the scores ring is
                        # never blocked across group boundaries.
                        if br == 0:
                            # keep unnormalized a1 and -lam*r1 for branch 2
                            # (a1u stays f32: tensor_mul encodes one source
                            # dtype, so mixing bf16/f32 inputs corrupts data)
                            a1u = kp.tile([P, 2, NCH], F32, tag="a1u", name="a1u")
                            nc.vector.tensor_copy(a1u, pa)
                            r1l = kp.tile([P, 2, NCH], F32, tag="r1l", name="r1l")
                            for vh in range(2):
                                rpv = rmsp.tile([P, NCH], F32, tag="r", name="rpv")
                                nc.tensor.matmul(rpv, ones, racc[:, vh, :],
                                                 start=True, stop=True)
                                nc.vector.tensor_scalar_mul(r1l[:, vh, :], rpv, -lam)
                        else:
                            # o' = a1*r2 - lam*a2*r1 (per-column positive
                            # rescale of o; RMSNorm cancels it). m2 first: it
                            # reads pa, which gates the next group's pv.
                            m2 = ep.tile([P, 2, NCH], F32, tag="m2", name="m2")
                            nc.vector.tensor_mul(m2, pa, r1l)
                            m1 = ep.tile([P, 2, NCH], F32, tag="m1", name="m1")
                            for vh in range(2):
                                rpv = rmsp.tile([P, NCH], F32, tag="r", name="rpv")
                                nc.tensor.matmul(rpv, ones, racc[:, vh, :],
                                                 start=True, stop=True)
                                nc.vector.tensor_mul(m1[:, vh, :], a1u[:, vh, :], rpv)
                            opk = kp.tile([P, 2, NCH], F32, tag="opk", name="opk")
                            nc.vector.tensor_add(opk, m1, m2)
                    # per-head RMSNorm for this 512-column chunk:
                    # rsqrt(mean o'^2) = exp(-0.5*ln(mean)); Ln+Exp cost two
                    # ACT table swaps per group (different table sets).
                    sq = ep.tile([P, 2, NCH], BF16, tag="sq", name="sq")
                    nc.vector.tensor_mul(sq, opk, opk)
                    lnt = ep.tile([P, 2, NCH], F32, tag="ln", name="lnt")
                    for vh in range(2):
                        psm = rmsp.tile([P, NCH], F32, tag="r", name="psm")
                        nc.tensor.matmul(psm, ones, sq[:, vh, :],
                                         start=True, stop=True)
                        nc.scalar.activation(lnt[:, vh, :], psm, LOG, scale=1.0 / D2)
                    rsq = ep.tile([P, 2, NCH], F32, tag="rsq", name="rsq")
                    nc.scalar.activation(rsq, lnt, EXP, scale=-0.5)
                    on[q4] = kp.tile([P, 2, NCH], BF16, tag=f"on{q4}",
                                     name=f"on{q4}")
                    nc.vector.scalar_tensor_tensor(
                        on[q4], opk, sv, rsq, op0=MULT, op1=MULT)
                    emit_proj(q4)
                # final flush: 3-way buffer the proj psum through the
                # now-idle scores slots so the tail pipeline stays dense
                i = 0
                while pending:
                    pending.pop(0)(pool=psp if i % 3 else None)
                    i += 1
    nc.finalize()
    return nc


def _core_inputs(x, w_qkv, w_proj, rms_scale):
    """Host-side shard prep: per-core weight slices + replicated x^T (bf16)."""
    bf = ml_dtypes.bfloat16
    xt = np.ascontiguousarray(x.reshape(T, C).T)  # [C, T]
    xtr = np.ascontiguousarray(
        xt.reshape(KSLABS, P, T // NCH, NCH).transpose(1, 2, 0, 3)
    ).astype(bf)
    sv = np.ascontiguousarray(
        (rms_scale.astype(np.float32) * np.float32(1.0 - LAMBDA_INIT)).reshape(P, 1)
    )
    maps = []
    for c in range(N_CORES):
        cols = [
            w_qkv[:, 0 * 1024 + c * P:0 * 1024 + (c + 1) * P],  # q1 heads 2c,2c+1
            w_qkv[:, 1 * 1024 + c * P:1 * 1024 + (c + 1) * P],  # q2
            w_qkv[:, 2 * 1024 + c * P:2 * 1024 + (c + 1) * P],  # k1
            w_qkv[:, 3 * 1024 + c * P:3 * 1024 + (c + 1) * P],  # k2
        ]
        wqk = np.concatenate(cols, axis=1)  # [C, 512]
        wqk = np.ascontiguousarray(
            wqk.reshape(KSLABS, P, 4 * P).transpose(1, 0, 2)).astype(bf)
        wv = w_qkv[:, 2 * C + c * 2 * D2:2 * C + (c + 1) * 2 * D2]  # [C, 256]
        wv = np.ascontiguousarray(
            wv.reshape(KSLABS, P, 2 * D2).transpose(1, 0, 2)).astype(bf)
        wp = w_proj[c * 2 * D2:(c + 1) * 2 * D2, :]  # [256, T]
        wp = np.ascontiguousarray(
            wp.reshape(2, P, T).transpose(1, 0, 2)).astype(bf)
        maps.append({"xt": xtr, "wqk": wqk, "wv": wv, "wp": wp, "sv": sv})
    return maps


def kernel(x, w_qkv, w_proj, lambda_q1, lambda_k1, lambda_q2, lambda_k2, rms_scale):
    from concourse.bass_utils import run_bass_kernel_spmd

    x = np.asarray(x, dtype=np.float32)
    w_qkv = np.asarray(w_qkv, dtype=np.float32)
    w_proj = np.asarray(w_proj, dtype=np.float32)
    rms_scale = np.asarray(rms_scale, dtype=np.float32)
    lam1 = np.exp(np.sum(np.asarray(lambda_q1) * np.asarray(lambda_k1), dtype=np.float32))
    lam2 = np.exp(np.sum(np.asarray(lambda_q2) * np.asarray(lambda_k2), dtype=np.float32))
    lam = float(lam1 - lam2 + LAMBDA_INIT)

    nc = build(lam)
    in_maps = _core_inputs(x, w_qkv, w_proj, rms_scale)
    res = run_bass_kernel_spmd(nc, in_maps, core_ids=list(range(N_CORES)))
    y = np.zeros((TT, P, T), np.float32)
    for rmap in res.results:
        y += np.asarray(rmap["y"], dtype=np.float32)
    return y.reshape(1, T, C)


# revision 23
# speedup vs baseline: 1.1307x; 1.1307x over previous
"""Trainium2 Bass kernel for DiffSelfAttention (B=1, T=2048, C=2048, 16 v-heads).

Sharding: tensor-parallel over heads across 8 NeuronCores. Core c owns
v-heads {2c, 2c+1} plus the matching q/k heads of both differential branches.
Each core computes its qkv slice, the attention for its 4 q/k heads, the
differential + per-head RMSNorm, and a partial projection
y_c = out_c @ w_proj[rows_c]. The host sums the 8 partials (unshard step).

Layout/strategy notes (v3):
  - Everything that feeds the PE is bf16 (x, w_qkv slices, q/k/v, exp(scores),
    w_proj, normed out). PSUM accumulation stays fp32, so only input rounding
    (~0.1% rms) enters; measured end-to-end rel err ~2e-3 vs the 2e-2 budget.
    bf16 also enables Fast Weight Load on LDWEIGHTS and 2x DVE modes.
  - q/k produced transposed ([d, T]); v natural ([T, d]); scores computed
    transposed ([tk, tq]) so probs@v needs no transposes anywhere.
  - Softmax divisions eliminated: RMSNorm is invariant to per-column positive
    scale, so o' = a1*r2 - lam*a2*r1 replaces a1/r1 - lam*a2/r2.
  - The attention inner loop is software-pipelined: scores(k+1) is emitted on
    the PE before pv(k), so the PE streams scores while ACT exps slab k.
  - exp-sum r is accumulated on the DVE (bf16 adds over the et ring) with one
    final ones-matmul for the cross-partition reduction; this removes 131k PE
    cycles and frees 2 PSUM banks vs accumulating r on the PE.
  - RMSNorm (exp(-0.5*ln(mean)) on ACT) and the output projection run
    per-512-column group; the projection matmuls/copies/DMAs of group g are
    emitted one step per slab-beat inside group g+1, hiding them in the
    PE/ACT slack and spreading the output DMA across the whole kernel.
  - ~36 ones@ones warm-up matmuls at kernel start keep the PE busy while the
    first weight DMAs land, so the HAM clock-gate releases (1.2->2.4 GHz)
    right as real work begins. A dummy exp prepays the ACT table load.
"""

import math

import numpy as np
import ml_dtypes

import concourse.bass as bass
import concourse.bacc as bacc
import concourse.mybir as mybir
import concourse.tile as tile

F32 = mybir.dt.float32
F32R = mybir.dt.float32r
BF16 = mybir.dt.bfloat16

T = 2048
C = 2048
N_HEAD = 16
H_DIM = 64
D2 = 2 * H_DIM  # 128 (v-head dim, also the RMS group size)
LAMBDA_INIT = 0.8 - 0.6 * math.exp(-0.3)
SCALE = 1.0 / math.sqrt(H_DIM)
P = 128
KSLABS = C // P  # 16 contraction slabs
TT = T // P  # 16 t-tiles
NCH = 512  # tq chunk = PSUM bank width in fp32
N_CORES = 8
DMAG = 2  # k-slab granularity of the startup weight/x DMA stream
NWARM = 36  # HAM warm-up matmuls

EXP = mybir.ActivationFunctionType.Exp
LOG = mybir.ActivationFunctionType.Ln
MULT = mybir.AluOpType.mult


def build(lam: float) -> bass.Bass:
    nc = bacc.Bacc("TRN2", target_bir_lowering=False, debug=False)

    xt_d = nc.dram_tensor("xt", [P, 4, KSLABS, NCH], BF16, kind="ExternalInput")
    wqk_d = nc.dram_tensor("wqk", [P, KSLABS, 4 * P], BF16, kind="ExternalInput")
    wv_d = nc.dram_tensor("wv", [P, KSLABS, 2 * D2], BF16, kind="ExternalInput")
    wp_d = nc.dram_tensor("wp", [P, 2, T], BF16, kind="ExternalInput")
    sv_d = nc.dram_tensor("sv", [P, 1], F32, kind="ExternalInput")
    y_d = nc.dram_tensor("y", [TT, P, T], BF16, kind="ExternalOutput")

    with tile.TileContext(nc) as tc:
        with tc.tile_pool(name="persist", bufs=1) as persist:
            sv = persist.tile([P, 1], F32)
            ones_f = persist.tile([P, P], F32)
            ones = persist.tile([P, P], BF16)
            actw = persist.tile([P, 1], F32)
            qk = persist.tile([P, 4, T], BF16)  # q1|q2|k1|k2, [d, T] layout
            vnat = persist.tile([P, TT, 2 * D2], BF16)  # v, [T, d] layout
            nc.sync.dma_start(out=sv, in_=sv_d[:])
            nc.vector.memset(ones_f, 1.0)
            nc.vector.tensor_copy(ones, ones_f)
            # prepay the exp table load while the PE warms up
            nc.scalar.activation(actw, ones_f[:, 0:1], EXP, scale=1.0)

            # ---------- phase 1: qkv projections ----------
            with tc.tile_pool(name="w1", bufs=1) as w1p, \
                 tc.tile_pool(name="xt", bufs=2) as xtp, \
                 tc.tile_pool(name="ps_qk", bufs=2, space="PSUM") as pqk, \
                 tc.tile_pool(name="ps_v", bufs=2, space="PSUM") as pvp:
                warm = pqk.tile([P, P], F32, tag="warm", bufs=1)
                for _ in range(NWARM):
                    nc.tensor.matmul(warm, ones, ones, start=True, stop=True)
                wqk = w1p.tile([P, KSLABS, 4 * P], BF16)
                wv = w1p.tile([P, KSLABS, 2 * D2], BF16)
                for n in range(T // NCH):  # 512-wide t chunks
                    xt = xtp.tile([P, KSLABS, NCH], BF16)
                    if n == 0:
                        # Stream wqk + x chunk 0 in small interleaved k-slab
                        # groups so the first matmul starts a few us in (one
                        # big DMA each would stall the PE ~40us at start).
                        for g in range(KSLABS // DMAG):
                            sl = slice(g * DMAG, (g + 1) * DMAG)
                            nc.sync.dma_start(out=wqk[:, sl, :], in_=wqk_d[:, sl, :])
                            nc.sync.dma_start(out=xt[:, sl, :], in_=xt_d[:, 0, sl, :])
                        nc.sync.dma_start(out=wv, in_=wv_d[:])
                    else:
                        nc.sync.dma_start(out=xt, in_=xt_d[:, n, :, :])
                    # k-major accumulation into 4 live psum banks: each
                    # arriving x/w k-slab is consumed immediately, so the
                    # chunk-0 DMA stream never stalls a psum chain.
                    pss = [pqk.tile([P, NCH], F32, tag=f"m{m}", bufs=1,
                                    name=f"psqk{m}") for m in range(4)]
                    for k in range(KSLABS):
                        for m in range(4):  # q1, q2, k1, k2
                            nc.tensor.matmul(
                                pss[m],
                                wqk[:, k, m * P:(m + 1) * P],
                                xt[:, k, :],
                                start=(k == 0),
                                stop=(k == KSLABS - 1),
                            )
                    for m in range(4):
                        nc.vector.tensor_copy(qk[:, m, n * NCH:(n + 1) * NCH], pss[m])
                    for t2 in range(NCH // P):  # t-tiles in this chunk
                        ps = pvp.tile([P, 2 * D2], F32)
                        for k in range(KSLABS):
                            nc.tensor.matmul(
                                ps,
                                xt[:, k, t2 * P:(t2 + 1) * P],
                                wv[:, k, :],
                                start=(k == 0),
                                stop=(k == KSLABS - 1),
                            )
                        nc.vector.tensor_copy(vnat[:, n * (NCH // P) + t2, :], ps)

            # ---------- phases 2+3, fused per 512-column group ----------
            with tc.tile_pool(name="wp", bufs=1) as wpp, \
                 tc.tile_pool(name="ps_s", bufs=2, space="PSUM") as psp, \
                 tc.tile_pool(name="ps_a", bufs=1, space="PSUM") as pap, \
                 tc.tile_pool(name="ps_r", bufs=1, space="PSUM") as rmsp, \
                 tc.tile_pool(name="ps_y", bufs=1, space="PSUM") as pyp, \
                 tc.tile_pool(name="exp", bufs=2) as ep, \
                 tc.tile_pool(name="keep", bufs=1) as kp, \
                 tc.tile_pool(name="ysb", bufs=3) as yp:
                wp = wpp.tile([P, 2, T], BF16)
                nc.sync.dma_start(out=wp, in_=wp_d[:])

                # Deferred projection steps: emitted one per slab-beat so the
                # proj matmuls ride in the attention loop's dependency slack
                # instead of serializing between groups.
                pending = []

                def emit_proj(q4):
                    on_t = on[q4]
                    for t2 in range(NCH // P):
                        ysb = yp.tile([P, T], BF16, tag="ysb", name=f"ysb{q4}{t2}")
                        for nchk in range(T // NCH):
                            def step(t2=t2, nchk=nchk, ysb=ysb, on_t=on_t, q4=q4,
                                     pool=None):
                                if pool is None:
                                    py = pyp.tile([P, NCH], F32, name="py")
                                else:
                                    py = pool.tile([P, NCH], F32, tag="s", name="py")
                                for vh in range(2):
                                    nc.tensor.matmul(
                                        py,
                                        on_t[:, vh, t2 * P:(t2 + 1) * P],
                                        wp[:, vh, nchk * NCH:(nchk + 1) * NCH],
                                        start=(vh == 0),
                                        stop=(vh == 1),
                                    )
                                nc.vector.tensor_copy(
                                    ysb[:, nchk * NCH:(nchk + 1) * NCH], py)
                                if nchk == T // NCH - 1:
                                    nc.sync.dma_start(
                                        out=y_d[q4 * (NCH // P) + t2], in_=ysb)
                            pending.append(step)

                on = {}
                for q4 in range(4):
                    c0 = q4 * NCH
                    a1u = None
                    r1l = None
                    opk = None
                    for br in range(2):
                        pa = pap.tile([P, 2, NCH], F32, tag="pa", name="pa")
                        racc = kp.tile([P, 2, NCH], BF16, tag=f"racc{br}",
                                       name=f"racc{br}")

                        def scores(k):
                            ps = psp.tile([P, 2, NCH], F32, tag="s", name="ps")
                            for vh in range(2):
                                rows = slice(vh * H_DIM, (vh + 1) * H_DIM)
                                nc.tensor.matmul(
                                    ps[:, vh, :],
                                    qk[rows, 2 + br, k * P:(k + 1) * P],
                                    qk[rows, br, c0:c0 + NCH],
                                    start=True,
                                    stop=True,
                                )
                            return ps

                        ps_cur = scores(0)
                        for k in range(TT):  # tk slabs
                            ps_nxt = scores(k + 1) if k + 1 < TT else None
                            et = ep.tile([P, 2, NCH], BF16, tag="er", bufs=14,
                                         name="et")
                            nc.scalar.activation(et, ps_cur, EXP, scale=SCALE)
                            for vh in range(2):
                                nc.tensor.matmul(
                                    pa[:, vh, :],
                                    vnat[:, k, vh * D2:(vh + 1) * D2],
                                    et[:, vh, :],
                                    start=(k == 0),
                                    stop=(k == TT - 1),
                                )
                            if k == 0:
                                nc.vector.tensor_copy(racc, et)
                            else:
                                nc.vector.tensor_add(racc, racc, et)
                            if k % 2 == 1 and pending:
                                # one deferred proj step every other beat: the
                                # DVE can't absorb a copy every beat on top of
                                # the racc chain
                                pending.pop(0)()
                            ps_cur = ps_nxt
                        if br == 1:
                            # keep-alive matmuls at the group boundary: the PE
                            # otherwise idles >3.4us on the racc chain + ACT
                            # rms block, and the HAM clock-gate then throttles
                            # the next ~10us to 1.2 GHz
                            wt = rmsp.tile([P, NCH], F32, tag="r", name="wt")
                            for _ in range(8):
                                nc.tensor.matmul(wt[:, 0:P], ones, ones,
                                                 start=True, stop=True)
                        # cross-partition reduce of the exp-sums (broadcasts
                        # the column sums to all 128 partitions). Lives in the
                        # dedicated 1-bank rms pool so the scores ring is
                        # never blocked across group boundaries.
                        if br == 0:
                            # keep unnormalized a1 and -lam*r1 for branch 2
                            # (a1u stays f32: tensor_mul encodes one source
                            # dtype, so mixing bf16/f32 inputs corrupts data)
                            a1u = kp.tile([P, 2, NCH], F32, tag="a1u", name="a1u")
                            nc.vector.tensor_copy(a1u, pa)
                            r1l = kp.tile([P, 2, NCH], F32, tag="r1l", name="r1l")
                            for vh in range(2):
                                rpv = rmsp.tile([P, NCH], F32, tag="r", name="rpv")
                                nc.tensor.matmul(rpv, ones, racc[:, vh, :],
                                                 start=True, stop=True)
                                nc.vector.tensor_scalar_mul(r1l[:, vh, :], rpv, -lam)
                        else:
                            # o' = a1*r2 - lam*a2*r1 (per-column positive
                            # rescale of o; RMSNorm cancels it). m2 first: it
                            # reads pa, which gates the next group's pv.
                            m2 = ep.tile([P, 2, NCH], F32, tag="m2", name="m2")
                            nc.vector.tensor_mul(m2, pa, r1l)
                            m1 = ep.tile([P, 2, NCH], F32, tag="m1", name="m1")
                            for vh in range(2):
                                rpv = rmsp.tile([P, NCH], F32, tag="r", name="rpv")
                                nc.tensor.matmul(rpv, ones, racc[:, vh, :],
                                                 start=True, stop=True)
                                nc.vector.tensor_mul(m1[:, vh, :], a1u[:, vh, :], rpv)
                            opk = kp.tile([P, 2, NCH], F32, tag="opk", name="opk")
                            nc.vector.tensor_add(opk, m1, m2)
                    # per-head RMSNorm for this 512-column chunk:
                    # rsqrt(mean o'^2) = exp(-0.5*ln(mean)); Ln+Exp cost two
                    # ACT table swaps per group (different table sets).
                    sq = ep.tile([P, 2, NCH], BF16, tag="sq", name="sq")
                    nc.vector.tensor_mul(sq, opk, opk)
                    lnt = ep.tile([P, 2, NCH], F32, tag="ln", name="lnt")
                    for vh in range(2):
                        psm = rmsp.tile([P, NCH], F32, tag="r", name="psm")
                        nc.tensor.matmul(psm, ones, sq[:, vh, :],
                                         start=True, stop=True)
                        nc.scalar.activation(lnt[:, vh, :], psm, LOG, scale=1.0 / D2)
                    rsq = ep.tile([P, 2, NCH], F32, tag="rsq", name="rsq")
                    nc.scalar.activation(rsq, lnt, EXP, scale=-0.5)
                    on[q4] = kp.tile([P, 2, NCH], BF16, tag=f"on{q4}",
                                     name=f"on{q4}")
                    nc.vector.scalar_tensor_tensor(
                        on[q4], opk, sv, rsq, op0=MULT, op1=MULT)
                    emit_proj(q4)
                # final flush: 3-way buffer the proj psum through the
                # now-idle scores slots so the tail pipeline stays dense
                i = 0
                while pending:
                    pending.pop(0)(pool=psp if i % 3 else None)
                    i += 1
    nc.finalize()
    return nc


def _core_inputs(x, w_qkv, w_proj, rms_scale):
    """Host-side shard prep: per-core weight slices + replicated x^T (bf16)."""
    bf = ml_dtypes.bfloat16
    xt = np.ascontiguousarray(x.reshape(T, C).T)  # [C, T]
    xtr = np.ascontiguousarray(
        xt.reshape(KSLABS, P, T // NCH, NCH).transpose(1, 2, 0, 3)
    ).astype(bf)
    sv = np.ascontiguousarray(
        (rms_scale.astype(np.float32) * np.float32(1.0 - LAMBDA_INIT)).reshape(P, 1)
    )
    maps = []
    for c in range(N_CORES):
        cols = [
            w_qkv[:, 0 * 1024 + c * P:0 * 1024 + (c + 1) * P],  # q1 heads 2c,2c+1
            w_qkv[:, 1 * 1024 + c * P:1 * 1024 + (c + 1) * P],  # q2
            w_qkv[:, 2 * 1024 + c * P:2 * 1024 + (c + 1) * P],  # k1
            w_qkv[:, 3 * 1024 + c * P:3 * 1024 + (c + 1) * P],  # k2
        ]
        wqk = np.concatenate(cols, axis=1)  # [C, 512]
        wqk = np.ascontiguousarray(
            wqk.reshape(KSLABS, P, 4 * P).transpose(1, 0, 2)).astype(bf)
        wv = w_qkv[:, 2 * C + c * 2 * D2:2 * C + (c + 1) * 2 * D2]  # [C, 256]
        wv = np.ascontiguousarray(
            wv.reshape(KSLABS, P, 2 * D2).transpose(1, 0, 2)).astype(bf)
        wp = w_proj[c * 2 * D2:(c + 1) * 2 * D2, :]  # [256, T]
        wp = np.ascontiguousarray(
            wp.reshape(2, P, T).transpose(1, 0, 2)).astype(bf)
        maps.append({"xt": xtr, "wqk": wqk, "wv": wv, "wp": wp, "sv": sv})
    return maps


def kernel(x, w_qkv, w_proj, lambda_q1, lambda_k1, lambda_q2, lambda_k2, rms_scale):
    from concourse.bass_utils import run_bass_kernel_spmd

    x = np.asarray(x, dtype=np.float32)
    w_qkv = np.asarray(w_qkv, dtype=np.float32)
    w_proj = np.asarray(w_proj, dtype=np.float32)
    rms_scale = np.asarray(rms_scale, dtype=np.float32)
    lam1 = np.exp(np.sum(np.asarray(lambda_q1) * np.asarray(lambda_k1), dtype=np.float32))
    lam2 = np.exp(np.sum(np.asarray(lambda_q2) * np.asarray(lambda_k2), dtype=np.float32))
    lam = float(lam1 - lam2 + LAMBDA_INIT)

    nc = build(lam)
    in_maps = _core_inputs(x, w_qkv, w_proj, rms_scale)
    res = run_bass_kernel_spmd(nc, in_maps, core_ids=list(range(N_CORES)))
    y = np.zeros((TT, P, T), np.float32)
    for rmap in res.results:
        y += np.asarray(rmap["y"], dtype=np.float32)
    return y.reshape(1, T, C)
